# revision 1
# baseline (speedup 1.0000x reference)
"""Weighted Chamfer loss on Trainium2 (8 NeuronCores, batch-parallel).

Problem (per batch element b of 8):
    dist[i, j] = || set1[b, i] - set2[b, j] ||_2            (4096 x 4096, C=128)
    total = (sum_i w1[b,i] * min_j dist + sum_j w2[b,j] * min_i dist) / 2

Sharding: one batch element per NeuronCore (pure data parallel, no
collectives); the 8 per-core partial sums are added on the host.

Per-core pipeline, per [128 x 2048] PSUM unit (x row-block b, y col-half h):
  PE    : psum = x@y^T - x2/2 - y2/2 via fp16 matmuls (fp32 PSUM accum):
          4 main matmuls plus 4 rank-2 "bake" matmuls whose operands are
          zero-padded to K=128 (small-K weight loads are slow on HW);
          bake lhsT rows are [-x2/2 ; 1 ; 0...], rhs rows [1 ; -y2/2 ; 0...].
  ACT   : evacuates PSUM to SBUF fp16 with Identity(scale=-2) -> full d^2.
  DVE   : fp16 tensor_tensor(min) folds the block into the column-min
          accumulator (2x mode); a 2-level pairwise min fold + one strided
          tensor_reduce(min) produce the block row-min.
  Tail  : PE transposes of the column-min accumulator + min reduce give
          per-column mins; ACT relu+sqrt; DVE mul + sum-reduce for the
          weighted partials, summed on host.
"""

import sys
from contextlib import ExitStack, nullcontext

import numpy as np

for _p in ("/opt/trn_rl_repo",):
    if _p not in sys.path:
        sys.path.insert(0, _p)

import concourse.bass as bass
import concourse.tile as tile
from concourse import bacc, masks, mybir
from concourse.bass_utils import run_bass_kernel_spmd

AF = mybir.ActivationFunctionType
ALU = mybir.AluOpType
DT = mybir.dt

N_CORES = 8
N = 4096          # points per set per batch element
C = 128           # channels (= contraction dim = partition dim)
NB = N // 128     # 32 row blocks of x
UCOLS = 2048      # y columns per PSUM unit (half of PSUM)
NH = N // UCOLS   # 2 column halves
MMN = 512         # moving free dim per matmul (one fp32 PSUM bank)
NT = UCOLS // 128 # 16 transpose tiles per column half

_CACHE = {}
LAST_RESULTS = None  # BassKernelResults of the most recent run (for profiling)

def _build_program(repeat=1, parts="pe,act,dve"):
    nc = bacc.Bacc(
        "TRN2", debug=False, target_bir_lowering=False, num_devices=N_CORES
    )
    xt_d = nc.dram_tensor("xt", [C, N], DT.float32, kind="ExternalInput").ap()
    yt_d = nc.dram_tensor("yt", [C, N], DT.float32, kind="ExternalInput").ap()
    on_d = nc.dram_tensor("ones_row", [1, N], DT.float16, kind="ExternalInput").ap()
    w1t_d = nc.dram_tensor("w1t", [128, NB], DT.float32, kind="ExternalInput").ap()
    w2t_d = nc.dram_tensor("w2t", [128, NB], DT.float32, kind="ExternalInput").ap()
    out_d = nc.dram_tensor("out", [128, 2], DT.float32, kind="ExternalOutput").ap()

    en_act = "act" in parts
    en_dve = "dve" in parts

    with tile.TileContext(nc) as tc, ExitStack() as ctx:
        persist = ctx.enter_context(tc.tile_pool(name="persist", bufs=1))
        prep = ctx.enter_context(tc.tile_pool(name="prep", bufs=2))
        d2p = ctx.enter_context(tc.tile_pool(name="d2", bufs=8))
        psum = ctx.enter_context(tc.tile_pool(name="psum", bufs=2, space="PSUM"))

        # ---------------- inputs + fp16 casts ----------------
        xt = persist.tile([C, N], DT.float32)
        yt = persist.tile([C, N], DT.float32)
        nc.sync.dma_start(xt[:], xt_d[:])
        nc.sync.dma_start(yt[:], yt_d[:])
        w1t = persist.tile([128, NB], DT.float32)
        w2t = persist.tile([128, NB], DT.float32)
        nc.sync.dma_start(w1t[:], w1t_d[:])
        nc.sync.dma_start(w2t[:], w2t_d[:])

        xth = persist.tile([C, N], DT.float16)
        yth = persist.tile([C, N], DT.float16)
        nc.vector.tensor_copy(xth[:], xt[:])
        nc.vector.tensor_copy(yth[:], yt[:])

        identity = persist.tile([128, 128], DT.float16)
        masks.make_identity(nc, identity[:])

        ones = persist.tile([C, 1], DT.float16)
        nc.gpsimd.memset(ones[:], 1.0)

        colacc = persist.tile([128, N], DT.float16)
        nc.gpsimd.memset(colacc[:], 60000.0)

        rm = [
            persist.tile([128, NB], DT.float32, name=f"rm{i}", tag=f"rm{i}")
            for i in range(NH)
        ]
        colminT = persist.tile([128, NB], DT.float32)

        # bake operands, K padded to 128 with zero rows (small-K weight
        # loads measure ~500ns extra per matmul on HW):
        # lhsT rows [-x2/2 ; 1 ; 0...], rhs rows [1 ; -y2/2 ; 0...]
        bake_lhs = persist.tile([C, N], DT.float16)
        bake_rhs = persist.tile([C, N], DT.float16)
        nc.vector.memset(bake_lhs[:], 0.0)
        nc.vector.memset(bake_rhs[:], 0.0)
        nc.sync.dma_start(bake_lhs[1:2, :], on_d[:])
        nc.sync.dma_start(bake_rhs[0:1, :], on_d[:])

        # ---------------- squared norms ----------------
        # sq = t*t (ACT Square -> fp16), column-reduce over channels via
        # ones-matmul into PSUM row 0, then Identity(scale=-0.5) -> fp16 row;
        # x2 lands directly in bake_lhs row 0, y2 bounces via an SBUF->SBUF
        # DMA into bake_rhs row 1 (engines cannot write partition 1).
        norm_y = persist.tile([1, N], DT.float16)
        for src, dst, row in ((xt, bake_lhs, 0), (yt, norm_y, 1)):
            sq = prep.tile([C, N], DT.float16, tag="sq", name=f"sq{row}")
            nc.scalar.activation(sq[:], src[:], AF.Square)
            for half in range(NH):
                ps = psum.tile(
                    [128, UCOLS], DT.float32, tag="unit", name=f"nps{row}{half}"
                )
                for k in range(UCOLS // MMN):
                    c0 = k * MMN
                    nc.tensor.matmul(
                        ps[0:1, c0 : c0 + MMN],
                        ones[:],
                        sq[:, half * UCOLS + c0 : half * UCOLS + c0 + MMN],
                        start=True,
                        stop=True,
                    )
                nc.scalar.activation(
                    dst[0:1, half * UCOLS : (half + 1) * UCOLS],
                    ps[0:1, :],
                    AF.Identity,
                    scale=-0.5,
                )
        nc.sync.dma_start(bake_rhs[1:2, :], norm_y[:])

        if not en_dve:
            for t in rm:
                nc.gpsimd.memset(t[:], 1.0)

        with tc.For_i(0, repeat, 1) if repeat > 1 else nullcontext():
            # ---------------- main loop ----------------
            for h in range(NH):
                ycols = slice(h * UCOLS, (h + 1) * UCOLS)
                for b in range(NB):
                    ps = psum.tile([128, UCOLS], DT.float32, tag="unit")
                    for k in range(UCOLS // MMN):
                        c0 = k * MMN
                        nc.tensor.matmul(
                            ps[:, c0 : c0 + MMN],
                            xth[:, b * 128 : (b + 1) * 128],
                            yth[:, h * UCOLS + c0 : h * UCOLS + c0 + MMN],
                            start=True,
                            stop=False,
                        )
                    for k in range(UCOLS // MMN):
                        c0 = k * MMN
                        nc.tensor.matmul(
                            ps[:, c0 : c0 + MMN],
                            bake_lhs[:, b * 128 : (b + 1) * 128],
                            bake_rhs[:, h * UCOLS + c0 : h * UCOLS + c0 + MMN],
                            start=False,
                            stop=True,
                        )
                    d2 = d2p.tile([128, UCOLS], DT.float16, tag="d2")
                    if en_act:
                        if "esplit" in parts:
                            hw_ = UCOLS // 2
                            nc.scalar.activation(
                                d2[:, 0:hw_], ps[:, 0:hw_], AF.Identity, scale=-2.0
                            )
                            nc.scalar.activation(
                                d2[:, hw_:], ps[:, hw_:], AF.Identity, scale=-2.0
                            )
                        else:
                            nc.scalar.activation(
                                d2[:], ps[:], AF.Identity, scale=-2.0
                            )
                    if en_dve:
                        nc.vector.tensor_tensor(
                            colacc[:, ycols], d2[:], colacc[:, ycols], ALU.min
                        )
                        # pairwise fold, then one strided min-reduce
                        nfold = 2
                        for p in parts.split(","):
                            if p.startswith("fold"):
                                nfold = int(p[4:])
                        w = UCOLS // 2
                        for _ in range(nfold):
                            nc.vector.tensor_tensor(
                                d2[:, 0:w], d2[:, 0:w], d2[:, w : 2 * w], ALU.min
                            )
                            w //= 2
                        w *= 2
                        nc.vector.tensor_reduce(
                            rm[h][:, b : b + 1],
                            d2[:, 0:w].rearrange("p (t c) -> p t c", c=min(w, 512)),
                            axis=mybir.AxisListType.XY,
                            op=ALU.min,
                        )

            # column mins: PE-transpose 128-blocks of colacc into PSUM
            # units, then one strided min-reduce per half. Emitted after the
            # full loop so they never steal a main-unit PSUM slot mid-loop.
            for h in range(NH):
                pst = psum.tile([128, UCOLS], DT.float16, tag="unit", name="pst")
                for t in range(NT):
                    nc.tensor.transpose(
                        pst[:, t * 128 : (t + 1) * 128],
                        colacc[:, h * UCOLS + t * 128 : h * UCOLS + (t + 1) * 128],
                        identity[:],
                    )
                nc.vector.tensor_reduce(
                    colminT[:, h * NT : (h + 1) * NT],
                    pst[:].rearrange("p (t c) -> p t c", c=128),
                    axis=mybir.AxisListType.X,
                    op=ALU.min,
                )

            # ---------------- tail ----------------
            rowmin = persist.tile([128, NB], DT.float32)
            nc.vector.tensor_tensor(rowmin[:], rm[0][:], rm[1][:], ALU.min)

            rowr = persist.tile([128, NB], DT.float32)
            rowd = persist.tile([128, NB], DT.float32)
            nc.scalar.activation(rowr[:], rowmin[:], AF.Relu)
            nc.scalar.activation(rowd[:], rowr[:], AF.Sqrt)

            colr = persist.tile([128, NB], DT.float32)
            cold = persist.tile([128, NB], DT.float32)
            nc.scalar.activation(colr[:], colminT[:], AF.Relu)
            nc.scalar.activation(cold[:], colr[:], AF.Sqrt)

            junk = persist.tile([128, NB], DT.float32)
            outacc = persist.tile([128, 2], DT.float32)
            nc.vector.tensor_mul(junk[:], rowd[:], w1t[:])
            nc.vector.tensor_reduce(
                outacc[:, 0:1], junk[:], axis=mybir.AxisListType.X, op=ALU.add
            )
            junk2 = persist.tile([128, NB], DT.float32)
            nc.vector.tensor_mul(junk2[:], cold[:], w2t[:])
            nc.vector.tensor_reduce(
                outacc[:, 1:2], junk2[:], axis=mybir.AxisListType.X, op=ALU.add
            )
            nc.sync.dma_start(out_d[:], outacc[:])

    nc.compile()
    return nc


def _get_nc(repeat=1, parts="pe,act,dve"):
    key = ("nc", repeat, parts)
    if key not in _CACHE:
        _CACHE[key] = _build_program(repeat, parts)
    return _CACHE[key]


def _make_in_maps(set1, set2, w1, w2):
    ones_row = np.ones((1, N), dtype=np.float16)
    in_maps = []
    for b in range(N_CORES):
        in_maps.append(
            {
                "xt": np.ascontiguousarray(set1[b].T, dtype=np.float32),
                "yt": np.ascontiguousarray(set2[b].T, dtype=np.float32),
                "ones_row": ones_row,
                "w1t": np.ascontiguousarray(
                    w1[b].reshape(NB, 128).T, dtype=np.float32
                ),
                "w2t": np.ascontiguousarray(
                    w2[b].reshape(NB, 128).T, dtype=np.float32
                ),
            }
        )
    return in_maps


def kernel(set1, set2, w1, w2):
    global LAST_RESULTS
    set1 = np.asarray(set1, dtype=np.float32)
    set2 = np.asarray(set2, dtype=np.float32)
    w1 = np.asarray(w1, dtype=np.float32)
    w2 = np.asarray(w2, dtype=np.float32)

    nc = _get_nc()
    in_maps = _make_in_maps(set1, set2, w1, w2)
    res = run_bass_kernel_spmd(nc, in_maps, core_ids=list(range(N_CORES)))
    LAST_RESULTS = res

    total = 0.0
    for core_out in res.results:
        total += float(core_out["out"].astype(np.float64).sum())
    return np.float32(total / 2.0)



# revision 2
# speedup vs baseline: 1.0417x; 1.0417x over previous
"""Weighted Chamfer loss on Trainium2 (8 NeuronCores, batch-parallel).

Problem (per batch element b of 8):
    dist[i, j] = || set1[b, i] - set2[b, j] ||_2            (4096 x 4096, C=128)
    total = (sum_i w1[b,i] * min_j dist + sum_j w2[b,j] * min_i dist) / 2

Sharding: one batch element per NeuronCore (pure data parallel, no
collectives); the 8 per-core partial sums are added on the host.

Per-core pipeline (units are [128 x 2048] fp32 PSUM tiles; 32 row blocks
x 2 column halves):
  PE   : ONE fp8 DoubleRow matmul per 512-col chunk computes
         m = x.y - x2/2 - y2/2 over a K=256 contraction: k-tile 0 holds the
         128 channels (e4m3-cast inputs), k-tile 1 holds the bake rows
         (3 fp8 residual rows per squared-norm + matching ones rows).
         4.3 GFLOP at ~2x fp8 rate.
  ACT  : evacuates most units PSUM->SBUF fp16 with Identity(scale=-2) -> d2.
  DVE  : col pass = tensor_tensor(min) into colacc (fp16 2x mode);
         row pass = ONE tensor_scalar per row block over [128, 4096] with
         accum_out(op1=min) -> row-min in a single 4x-mode pass.
         For `dcopy` row blocks DVE evacuates instead of ACT, via
         tensor_scalar(mult -2) from PSUM whose accum_out is that half's
         row-min (rebalances the ACT-bound steady state).
  Tail : PE transposes of colacc + strided min-reduce give per-column mins;
         ACT relu+sqrt; DVE mul + sum-reduce for weighted partials; host sum.
"""

import sys
from contextlib import ExitStack, nullcontext

import numpy as np

for _p in ("/opt/trn_rl_repo",):
    if _p not in sys.path:
        sys.path.insert(0, _p)

import concourse.bass as bass
import concourse.tile as tile
from concourse import bacc, masks, mybir
from concourse.bass_utils import run_bass_kernel_spmd

AF = mybir.ActivationFunctionType
ALU = mybir.AluOpType
DT = mybir.dt
PM = mybir.MatmulPerfMode

N_CORES = 8
N = 4096          # points per set per batch element
C = 128           # channels (= contraction dim = partition dim)
NB = N // 128     # 32 row blocks of x
UCOLS = 2048      # y columns per PSUM unit (half of PSUM)
NH = N // UCOLS   # 2 column halves
MMN = 512         # moving free dim per matmul (one fp32 PSUM bank)
NT = UCOLS // 128 # 16 transpose tiles per column half
NRES = 3          # fp8 residual rows per squared-norm bake

_CACHE = {}
LAST_RESULTS = None  # BassKernelResults of the most recent run (for profiling)

DEFAULT_PARTS = "pe,act,dve,dcopy2"


def _build_program(repeat=1, parts=DEFAULT_PARTS):
    en_act = "act" in parts
    en_dve = "dve" in parts
    ncopy = 0
    for p in parts.split(","):
        if p.startswith("dcopy"):
            ncopy = int(p[5:])
    # dcopy blocks spread evenly through the 32 row blocks, first at ~mid gap
    dset = set()
    if ncopy:
        step = NB // ncopy
        dset = {step // 2 + k * step for k in range(ncopy)}

    nc = bacc.Bacc(
        "TRN2", debug=False, target_bir_lowering=False, num_devices=N_CORES
    )
    xt_d = nc.dram_tensor("xt", [C, N], DT.float32, kind="ExternalInput").ap()
    yt_d = nc.dram_tensor("yt", [C, N], DT.float32, kind="ExternalInput").ap()
    w1t_d = nc.dram_tensor("w1t", [128, NB], DT.float32, kind="ExternalInput").ap()
    w2t_d = nc.dram_tensor("w2t", [128, NB], DT.float32, kind="ExternalInput").ap()
    out_d = nc.dram_tensor("out", [128, 2], DT.float32, kind="ExternalOutput").ap()

    with tile.TileContext(nc) as tc, ExitStack() as ctx:
        persist = ctx.enter_context(tc.tile_pool(name="persist", bufs=1))
        prep = ctx.enter_context(tc.tile_pool(name="prep", bufs=2))
        d2p = ctx.enter_context(tc.tile_pool(name="d2", bufs=3))
        psum = ctx.enter_context(tc.tile_pool(name="psum", bufs=2, space="PSUM"))

        # ---------------- inputs ----------------
        xt = persist.tile([C, N], DT.float32)
        yt = persist.tile([C, N], DT.float32)
        nc.sync.dma_start(xt[:], xt_d[:])
        nc.sync.dma_start(yt[:], yt_d[:])
        w1t = persist.tile([128, NB], DT.float32)
        w2t = persist.tile([128, NB], DT.float32)
        nc.sync.dma_start(w1t[:], w1t_d[:])
        nc.sync.dma_start(w2t[:], w2t_d[:])

        # fused fp8 operands: k-tile 0 = channels, k-tile 1 = bake rows
        x8 = persist.tile([C, 2, N], DT.float8e4)
        y8 = persist.tile([C, 2, N], DT.float8e4)
        nc.vector.memset(x8[:], 0.0)
        nc.vector.memset(y8[:], 0.0)
        nc.vector.tensor_copy(x8[:, 0, :], xt[:])
        nc.vector.tensor_copy(y8[:, 0, :], yt[:])

        identity = persist.tile([128, 128], DT.float16)
        masks.make_identity(nc, identity[:])

        ones = persist.tile([C, 1], DT.float16)
        nc.gpsimd.memset(ones[:], 1.0)

        colacc = persist.tile([128, N], DT.float16)
        nc.gpsimd.memset(colacc[:], 60000.0)

        rm = persist.tile([128, NB], DT.float32)
        rmh0 = persist.tile([128, NB], DT.float32)
        rmh1 = persist.tile([128, NB], DT.float32)
        junk = persist.tile([128, N], DT.float16)

        # ---------------- squared norms -> fp8 residual bake rows ----------
        # v = -x2/2 as a [1, N] fp32 row via Square + ones-matmul +
        # Identity(scale=-0.5); then 3 e4m3 residual rows DMA'd into
        # k-tile 1 partitions (x side rows 0-2 pair with ones in y8 rows 0-2,
        # y side rows 3-5 pair with ones in x8 rows 3-5).
        on8 = persist.tile([1, N], DT.float8e4)
        nc.vector.memset(on8[:], 1.0)
        for src, targ, pbase, oth in ((xt, x8, 0, y8), (yt, y8, NRES, x8)):
            sq = prep.tile([C, N], DT.float16, tag="sq", name=f"sq{pbase}")
            nc.scalar.activation(sq[:], src[:], AF.Square)
            v = prep.tile([1, N], DT.float32, tag="v", name=f"v{pbase}")
            for half in range(NH):
                ps = psum.tile(
                    [128, UCOLS], DT.float32, tag="unit", name=f"nps{pbase}{half}"
                )
                for k in range(UCOLS // MMN):
                    c0 = k * MMN
                    nc.tensor.matmul(
                        ps[0:1, c0 : c0 + MMN],
                        ones[:],
                        sq[:, half * UCOLS + c0 : half * UCOLS + c0 + MMN],
                        start=True,
                        stop=True,
                    )
                nc.scalar.activation(
                    v[0:1, half * UCOLS : (half + 1) * UCOLS],
                    ps[0:1, :],
                    AF.Identity,
                    scale=-0.5,
                )
            cur = v
            for k in range(NRES):
                r8 = prep.tile([1, N], DT.float8e4, tag="r8", name=f"r8{pbase}{k}")
                nc.vector.tensor_copy(r8[:], cur[:])
                nc.sync.dma_start(targ[pbase + k : pbase + k + 1, 1, :], r8[:])
                nc.sync.dma_start(oth[pbase + k : pbase + k + 1, 1, :], on8[:])
                if k < NRES - 1:
                    rb = prep.tile([1, N], DT.float32, tag="rb", name=f"rb{pbase}{k}")
                    nc.vector.tensor_copy(rb[:], r8[:])
                    nxt = prep.tile(
                        [1, N], DT.float32, tag="v", name=f"v{pbase}{k}"
                    )
                    nc.vector.tensor_tensor(nxt[:], cur[:], rb[:], ALU.subtract)
                    cur = nxt

        if not en_dve:
            nc.gpsimd.memset(rm[:], 1.0)

        with tc.For_i(0, repeat, 1) if repeat > 1 else nullcontext():
            # ---------------- main loop ----------------
            for b in range(NB):
                bcols = slice(b * 128, (b + 1) * 128)
                d2 = d2p.tile([128, N], DT.float16, tag="d2")
                dvecopy = b in dset and en_dve
                for h in range(NH):
                    hcols = slice(h * UCOLS, (h + 1) * UCOLS)
                    ps = psum.tile([128, UCOLS], DT.float32, tag="unit")
                    for k in range(UCOLS // MMN):
                        c0 = k * MMN
                        nc.tensor.matmul(
                            ps[:, c0 : c0 + MMN],
                            x8[:, :, bcols],
                            y8[:, :, h * UCOLS + c0 : h * UCOLS + c0 + MMN],
                            start=True,
                            stop=True,
                            perf_mode=PM.DoubleRow,
                        )
                    if dvecopy:
                        # DVE evac (PSUM fp32 -> SBUF fp16, scale -2) whose
                        # accum is this half's row-min
                        rmh = rmh0 if h == 0 else rmh1
                        nc.vector.tensor_scalar(
                            d2[:, hcols],
                            ps[:],
                            -2.0,
                            None,
                            ALU.mult,
                            ALU.min,
                            accum_out=rmh[:, b : b + 1],
                        )
                    elif en_act:
                        nc.scalar.activation(
                            d2[:, hcols], ps[:], AF.Identity, scale=-2.0
                        )
                    if en_dve and en_act:
                        nc.vector.tensor_tensor(
                            colacc[:, hcols], d2[:, hcols], colacc[:, hcols], ALU.min
                        )
                if en_dve and en_act:
                    if dvecopy:
                        nc.vector.tensor_tensor(
                            rm[:, b : b + 1],
                            rmh0[:, b : b + 1],
                            rmh1[:, b : b + 1],
                            ALU.min,
                        )
                    else:
                        nc.vector.tensor_scalar(
                            junk[:],
                            d2[:],
                            1.0,
                            None,
                            ALU.mult,
                            ALU.min,
                            accum_out=rm[:, b : b + 1],
                        )

            # column mins: PE-transpose 128-blocks of colacc into PSUM
            # units, then one strided min-reduce per half. Emitted after the
            # full loop so they never steal a main-unit PSUM slot mid-loop.
            colminT = persist.tile([128, NB], DT.float32)
            for h in range(NH):
                pst = psum.tile([128, UCOLS], DT.float16, tag="unit", name="pst")
                for t in range(NT):
                    nc.tensor.transpose(
                        pst[:, t * 128 : (t + 1) * 128],
                        colacc[:, h * UCOLS + t * 128 : h * UCOLS + (t + 1) * 128],
                        identity[:],
                    )
                nc.vector.tensor_reduce(
                    colminT[:, h * NT : (h + 1) * NT],
                    pst[:].rearrange("p (t c) -> p t c", c=128),
                    axis=mybir.AxisListType.X,
                    op=ALU.min,
                )

            # ---------------- tail ----------------
            rowr = persist.tile([128, NB], DT.float32)
            rowd = persist.tile([128, NB], DT.float32)
            nc.scalar.activation(rowr[:], rm[:], AF.Relu)
            nc.scalar.activation(rowd[:], rowr[:], AF.Sqrt)

            colr = persist.tile([128, NB], DT.float32)
            cold = persist.tile([128, NB], DT.float32)
            nc.scalar.activation(colr[:], colminT[:], AF.Relu)
            nc.scalar.activation(cold[:], colr[:], AF.Sqrt)

            junk1 = persist.tile([128, NB], DT.float32)
            outacc = persist.tile([128, 2], DT.float32)
            nc.vector.tensor_mul(junk1[:], rowd[:], w1t[:])
            nc.vector.tensor_reduce(
                outacc[:, 0:1], junk1[:], axis=mybir.AxisListType.X, op=ALU.add
            )
            junk2 = persist.tile([128, NB], DT.float32)
            nc.vector.tensor_mul(junk2[:], cold[:], w2t[:])
            nc.vector.tensor_reduce(
                outacc[:, 1:2], junk2[:], axis=mybir.AxisListType.X, op=ALU.add
            )
            nc.sync.dma_start(out_d[:], outacc[:])

    nc.compile()
    return nc


def _get_nc(repeat=1, parts=DEFAULT_PARTS):
    key = ("nc", repeat, parts)
    if key not in _CACHE:
        _CACHE[key] = _build_program(repeat, parts)
    return _CACHE[key]


def _make_in_maps(set1, set2, w1, w2):
    in_maps = []
    for b in range(N_CORES):
        in_maps.append(
            {
                "xt": np.ascontiguousarray(set1[b].T, dtype=np.float32),
                "yt": np.ascontiguousarray(set2[b].T, dtype=np.float32),
                "w1t": np.ascontiguousarray(
                    w1[b].reshape(NB, 128).T, dtype=np.float32
                ),
                "w2t": np.ascontiguousarray(
                    w2[b].reshape(NB, 128).T, dtype=np.float32
                ),
            }
        )
    return in_maps


def kernel(set1, set2, w1, w2):
    global LAST_RESULTS
    set1 = np.asarray(set1, dtype=np.float32)
    set2 = np.asarray(set2, dtype=np.float32)
    w1 = np.asarray(w1, dtype=np.float32)
    w2 = np.asarray(w2, dtype=np.float32)

    nc = _get_nc()
    in_maps = _make_in_maps(set1, set2, w1, w2)
    res = run_bass_kernel_spmd(nc, in_maps, core_ids=list(range(N_CORES)))
    LAST_RESULTS = res

    total = 0.0
    for core_out in res.results:
        total += float(core_out["out"].astype(np.float64).sum())
    return np.float32(total / 2.0)


# revision 13
# speedup vs baseline: 1.0837x; 1.0404x over previous
"""Weighted Chamfer loss on Trainium2 (8 NeuronCores, batch-parallel).

Problem (per batch element b of 8):
    dist[i, j] = || set1[b, i] - set2[b, j] ||_2            (4096 x 4096, C=128)
    total = (sum_i w1[b,i] * min_j dist + sum_j w2[b,j] * min_i dist) / 2

Sharding: one batch element per NeuronCore (pure data parallel, no
collectives); the 8 per-core partial sums are added on the host.

Per-core pipeline (units are [128 x 2048] fp32 PSUM tiles; 32 row blocks
x 2 column halves):
  PE  : ONE fp8 DoubleRow matmul per 512-col chunk computes
        m = x.y - x2/2 - y2/2 (= -d2/2) over a K=256 contraction:
        k-tile 0 = the 128 channels (e4m3-cast inputs), k-tile 1 = bake
        rows (3 fp8 residual rows per squared-norm + matching ones rows).
  exp mode (default): evacuation carries the row reduction.
    ACT : evacuates E-blocks as exp(-beta*(d2 - REF)) in bf16
          (Exp(scale=2*beta, bias=beta*REF) of the PSUM); its built-in
          sum accumulator emits per-unit softmin sums -> row-min comes
          out of the evac pass for free (rowmin = REF - ln(S)/beta).
    DVE : col pass only = tensor_tensor(max) of the (monotone) exps into
          colacc; for `dcopy` D-blocks DVE instead evacuates from PSUM
          via tensor_scalar(mult -2 -> fp16 d2) whose accum(min) is that
          half's row-min, and folds into a separate fp16 min-colacc
          (rebalances ACT vs DVE).
    Tail: PE transposes of both col accumulators + strided max/min
          reduces; Ln/Relu/Sqrt on ACT; masked merge of the two row/col
          conventions; weighted sums; host adds the 8 partials.
"""

import sys
from contextlib import ExitStack, nullcontext

import numpy as np

for _p in ("/opt/trn_rl_repo",):
    if _p not in sys.path:
        sys.path.insert(0, _p)

import concourse.bass as bass
import concourse.tile as tile
from concourse import bacc, masks, mybir
from concourse.bass_utils import run_bass_kernel_spmd

AF = mybir.ActivationFunctionType
ALU = mybir.AluOpType
DT = mybir.dt
PM = mybir.MatmulPerfMode

N_CORES = 8
N = 4096          # points per set per batch element
C = 128           # channels (= contraction dim = partition dim)
NB = N // 128     # 32 row blocks of x
UCOLS = 2048      # y columns per PSUM unit (half of PSUM)
NH = N // UCOLS   # 2 column halves
MMN = 512         # moving free dim per matmul (one fp32 PSUM bank)
NT = UCOLS // 128 # 16 transpose tiles per column half
NRES = 3          # fp8 residual rows per squared-norm bake

BETA = 0.75       # softmin sharpness (exp mode)
REF = 150.0       # softmin reference offset (exp mode)

_CACHE = {}
LAST_RESULTS = None  # BassKernelResults of the most recent run (for profiling)

DEFAULT_PARTS = "pe,act,dve,exp,dcopy8"


def _build_program(repeat=1, parts=DEFAULT_PARTS):
    en_act = "act" in parts
    en_dve = "dve" in parts
    en_col = "nocol" not in parts
    en_row = "norow" not in parts
    rowttr = "rowttr" in parts
    en_exp = "exp" in parts
    mmn = MMN
    ncopy = 0
    for p in parts.split(","):
        if p.startswith("dcopy"):
            ncopy = int(p[5:])
        if p.startswith("mmn"):
            mmn = int(p[3:])
    # dcopy blocks spread evenly through the 32 row blocks
    dset = set()
    if ncopy:
        step = NB // ncopy
        dset = {step // 2 + k * step for k in range(ncopy)}

    nc = bacc.Bacc(
        "TRN2", debug=False, target_bir_lowering=False, num_devices=N_CORES
    )
    xt_d = nc.dram_tensor("xt", [C, N], DT.float32, kind="ExternalInput").ap()
    yt_d = nc.dram_tensor("yt", [C, N], DT.float32, kind="ExternalInput").ap()
    w1t_d = nc.dram_tensor("w1t", [128, NB], DT.float32, kind="ExternalInput").ap()
    w2t_d = nc.dram_tensor("w2t", [128, NB], DT.float32, kind="ExternalInput").ap()
    out_d = nc.dram_tensor("out", [128, 2], DT.float32, kind="ExternalOutput").ap()

    with tile.TileContext(nc) as tc, ExitStack() as ctx:
        persist = ctx.enter_context(tc.tile_pool(name="persist", bufs=1))
        prep = ctx.enter_context(tc.tile_pool(name="prep", bufs=2))
        d2p = ctx.enter_context(tc.tile_pool(name="d2", bufs=3))
        d2fp = ctx.enter_context(tc.tile_pool(name="d2f", bufs=1))
        psum = ctx.enter_context(tc.tile_pool(name="psum", bufs=2, space="PSUM"))

        # ---------------- inputs ----------------
        xt = persist.tile([C, N], DT.float32)
        yt = persist.tile([C, N], DT.float32)
        nc.sync.dma_start(xt[:], xt_d[:])
        nc.sync.dma_start(yt[:], yt_d[:])
        w1t = persist.tile([128, NB], DT.float32)
        w2t = persist.tile([128, NB], DT.float32)
        nc.sync.dma_start(w1t[:], w1t_d[:])
        nc.sync.dma_start(w2t[:], w2t_d[:])

        # fused fp8 operands: k-tile 0 = channels, k-tile 1 = bake rows
        x8 = persist.tile([C, 2, N], DT.float8e4)
        y8 = persist.tile([C, 2, N], DT.float8e4)
        nc.vector.memset(x8[:], 0.0)
        nc.vector.memset(y8[:], 0.0)
        nc.vector.tensor_copy(x8[:, 0, :], xt[:])
        nc.vector.tensor_copy(y8[:, 0, :], yt[:])

        identity = persist.tile([128, 128], DT.float16)
        masks.make_identity(nc, identity[:])
        identb = persist.tile([128, 128], DT.bfloat16)
        nc.vector.tensor_copy(identb[:], identity[:])

        ones = persist.tile([C, 1], DT.float16)
        nc.gpsimd.memset(ones[:], 1.0)

        # exp-domain (bf16, max-acc) and d2-domain (fp16, min-acc) col accs
        colacc = persist.tile([128, N], DT.bfloat16 if en_exp else DT.float16)
        nc.gpsimd.memset(colacc[:], 0.0 if en_exp else 60000.0)
        colacc2 = None
        if en_exp and ncopy:
            colacc2 = persist.tile([128, N], DT.float16)
            nc.gpsimd.memset(colacc2[:], 60000.0)

        bref = persist.tile([128, 1], DT.float32)
        nc.gpsimd.memset(bref[:], BETA * REF)
        refc = persist.tile([128, 1], DT.float32)
        nc.gpsimd.memset(refc[:], REF)

        rm = persist.tile([128, NB], DT.float32)
        rmh0 = persist.tile([128, NB], DT.float32)
        rmh1 = persist.tile([128, NB], DT.float32)
        se0 = persist.tile([128, NB], DT.float32)
        se1 = persist.tile([128, NB], DT.float32)
        junk = None
        if not en_exp:
            junk = persist.tile([128, N], DT.float16)

        # D-column masks (exp mode): dm = 1 on dcopy columns, em = 1 - dm
        dm = persist.tile([128, NB], DT.float32)
        em = persist.tile([128, NB], DT.float32)
        nc.gpsimd.memset(dm[:], 0.0)
        nc.gpsimd.memset(em[:], 1.0)
        for b in sorted(dset):
            nc.gpsimd.memset(dm[:, b : b + 1], 1.0)
            nc.gpsimd.memset(em[:, b : b + 1], 0.0)

        # ---------------- squared norms -> fp8 residual bake rows ----------
        on8 = persist.tile([1, N], DT.float8e4)
        nc.vector.memset(on8[:], 1.0)
        for src, targ, pbase, oth in ((xt, x8, 0, y8), (yt, y8, NRES, x8)):
            sq = prep.tile([C, N], DT.float16, tag="sq", name=f"sq{pbase}")
            nc.scalar.activation(sq[:], src[:], AF.Square)
            v = prep.tile([1, N], DT.float32, tag="v", name=f"v{pbase}")
            for half in range(NH):
                ps = psum.tile(
                    [128, UCOLS], DT.float32, tag="unit", name=f"nps{pbase}{half}"
                )
                for k in range(UCOLS // MMN):
                    c0 = k * MMN
                    nc.tensor.matmul(
                        ps[0:1, c0 : c0 + MMN],
                        ones[:],
                        sq[:, half * UCOLS + c0 : half * UCOLS + c0 + MMN],
                        start=True,
                        stop=True,
                    )
                nc.scalar.activation(
                    v[0:1, half * UCOLS : (half + 1) * UCOLS],
                    ps[0:1, :],
                    AF.Identity,
                    scale=-0.5,
                )
            cur = v
            for k in range(NRES):
                r8 = prep.tile([1, N], DT.float8e4, tag="r8", name=f"r8{pbase}{k}")
                nc.vector.tensor_copy(r8[:], cur[:])
                nc.sync.dma_start(targ[pbase + k : pbase + k + 1, 1, :], r8[:])
                nc.sync.dma_start(oth[pbase + k : pbase + k + 1, 1, :], on8[:])
                if k < NRES - 1:
                    rb = prep.tile([1, N], DT.float32, tag="rb", name=f"rb{pbase}{k}")
                    nc.vector.tensor_copy(rb[:], r8[:])
                    nxt = prep.tile(
                        [1, N], DT.float32, tag="v", name=f"v{pbase}{k}"
                    )
                    nc.vector.tensor_tensor(nxt[:], cur[:], rb[:], ALU.subtract)
                    cur = nxt

        if not (en_dve and en_act and en_row):
            nc.gpsimd.memset(rm[:], 1.0)

        unroll = "unroll" in parts
        with (
            tc.For_i(0, repeat, 1)
            if (repeat > 1 and not unroll)
            else nullcontext()
        ):
          for _u in range(repeat if unroll else 1):
            # per-iteration neutral fill for partial-column accumulators
            if en_exp:
                nc.gpsimd.memset(se0[:], 1.0)
                nc.gpsimd.memset(se1[:], 1.0)
                if ncopy:
                    nc.gpsimd.memset(rmh0[:], 1.0)
                    nc.gpsimd.memset(rmh1[:], 1.0)
            # ---------------- main loop ----------------
            for b in range(NB):
                bcols = slice(b * 128, (b + 1) * 128)
                dvecopy = b in dset and en_dve
                if dvecopy:
                    d2 = d2fp.tile([128, N], DT.float16, tag="d2f", name="d2f")
                else:
                    d2 = d2p.tile(
                        [128, N], DT.bfloat16 if en_exp else DT.float16, tag="d2"
                    )
                for h in range(NH):
                    hcols = slice(h * UCOLS, (h + 1) * UCOLS)
                    ps = psum.tile([128, UCOLS], DT.float32, tag="unit")
                    for k in range(UCOLS // mmn):
                        c0 = k * mmn
                        nc.tensor.matmul(
                            ps[:, c0 : c0 + mmn],
                            x8[:, :, bcols],
                            y8[:, :, h * UCOLS + c0 : h * UCOLS + c0 + mmn],
                            start=True,
                            stop=True,
                            perf_mode=PM.DoubleRow,
                        )
                    rmh = rmh0 if h == 0 else rmh1
                    seh = se0 if h == 0 else se1
                    if dvecopy:
                        # DVE evac (PSUM fp32 -> SBUF fp16 d2, scale -2);
                        # accum is this half's row-min
                        nc.vector.tensor_scalar(
                            d2[:, hcols],
                            ps[:],
                            -2.0,
                            None,
                            ALU.mult,
                            ALU.min,
                            accum_out=rmh[:, b : b + 1],
                        )
                        if en_col:
                            nc.vector.tensor_tensor(
                                colacc2[:, hcols],
                                d2[:, hcols],
                                colacc2[:, hcols],
                                ALU.min,
                            )
                    elif en_act:
                        if en_exp:
                            # evac as exp(-beta*(d2-REF)); accum = softmin sum
                            nc.scalar.activation(
                                d2[:, hcols],
                                ps[:],
                                AF.Exp,
                                bias=bref[:],
                                scale=2.0 * BETA,
                                accum_out=seh[:, b : b + 1] if en_row else None,
                            )
                            if en_dve and en_col:
                                nc.vector.tensor_tensor(
                                    colacc[:, hcols],
                                    d2[:, hcols],
                                    colacc[:, hcols],
                                    ALU.max,
                                )
                        else:
                            nc.scalar.activation(
                                d2[:, hcols], ps[:], AF.Identity, scale=-2.0
                            )
                            if en_dve and en_col:
                                nc.vector.tensor_tensor(
                                    colacc[:, hcols],
                                    d2[:, hcols],
                                    colacc[:, hcols],
                                    ALU.min,
                                )
                if en_dve and en_act and en_row and not en_exp and not dvecopy:
                    if rowttr:
                        nc.vector.tensor_tensor(
                            junk[:, 0:UCOLS], d2[:, 0:UCOLS], d2[:, UCOLS:N], ALU.min
                        )
                        nc.vector.tensor_tensor(
                            junk[:, 0:1024], junk[:, 0:1024], junk[:, 1024:UCOLS], ALU.min
                        )
                        nc.vector.tensor_tensor(
                            junk[:, 0:512], junk[:, 0:512], junk[:, 512:1024], ALU.min
                        )
                        nc.vector.tensor_reduce(
                            rm[:, b : b + 1],
                            junk[:, 0:512],
                            axis=mybir.AxisListType.X,
                            op=ALU.min,
                        )
                    else:
                        nc.vector.tensor_scalar(
                            junk[:],
                            d2[:],
                            1.0,
                            None,
                            ALU.mult,
                            ALU.min,
                            accum_out=rm[:, b : b + 1],
                        )
                elif en_dve and en_act and en_row and not en_exp and dvecopy:
                    nc.vector.tensor_tensor(
                        rm[:, b : b + 1],
                        rmh0[:, b : b + 1],
                        rmh1[:, b : b + 1],
                        ALU.min,
                    )

            # ---------------- column-min tails ----------------
            # exp-domain: transpose colacc (bf16) + strided MAX reduce
            colminT = persist.tile([128, NB], DT.float32)
            for h in range(NH):
                pst = psum.tile(
                    [128, UCOLS],
                    DT.bfloat16 if en_exp else DT.float16,
                    tag="unit",
                    name="pst",
                )
                for t in range(NT):
                    nc.tensor.transpose(
                        pst[:, t * 128 : (t + 1) * 128],
                        colacc[:, h * UCOLS + t * 128 : h * UCOLS + (t + 1) * 128],
                        identb[:] if en_exp else identity[:],
                    )
                nc.vector.tensor_reduce(
                    colminT[:, h * NT : (h + 1) * NT],
                    pst[:].rearrange("p (t c) -> p t c", c=128),
                    axis=mybir.AxisListType.X,
                    op=ALU.max if en_exp else ALU.min,
                )
            colminT2 = None
            if en_exp and ncopy:
                colminT2 = persist.tile([128, NB], DT.float32)
                for h in range(NH):
                    pst2 = psum.tile(
                        [128, UCOLS], DT.float16, tag="unit", name="pst2"
                    )
                    for t in range(NT):
                        nc.tensor.transpose(
                            pst2[:, t * 128 : (t + 1) * 128],
                            colacc2[
                                :, h * UCOLS + t * 128 : h * UCOLS + (t + 1) * 128
                            ],
                            identity[:],
                        )
                    nc.vector.tensor_reduce(
                        colminT2[:, h * NT : (h + 1) * NT],
                        pst2[:].rearrange("p (t c) -> p t c", c=128),
                        axis=mybir.AxisListType.X,
                        op=ALU.min,
                    )

            # ---------------- tail ----------------
            if en_exp:
                # rows: REF - ln(se0+se1)/beta on E columns, min(rmh) on D
                sadd = persist.tile([128, NB], DT.float32)
                nc.vector.tensor_tensor(sadd[:], se0[:], se1[:], ALU.add)
                lrow = persist.tile([128, NB], DT.float32)
                nc.scalar.activation(lrow[:], sadd[:], AF.Ln)
                rmexp = persist.tile([128, NB], DT.float32)
                nc.scalar.activation(
                    rmexp[:], lrow[:], AF.Identity, scale=-1.0 / BETA, bias=refc[:]
                )
                if ncopy:
                    rmd = persist.tile([128, NB], DT.float32)
                    nc.vector.tensor_tensor(rmd[:], rmh0[:], rmh1[:], ALU.min)
                    t1 = persist.tile([128, NB], DT.float32)
                    t2 = persist.tile([128, NB], DT.float32)
                    nc.vector.tensor_mul(t1[:], rmexp[:], em[:])
                    nc.vector.tensor_mul(t2[:], rmd[:], dm[:])
                    nc.vector.tensor_tensor(rm[:], t1[:], t2[:], ALU.add)
                else:
                    nc.vector.tensor_copy(rm[:], rmexp[:])
                # cols: REF - ln(colmax)/beta, merged with fp16 min-colacc
                lcol = persist.tile([128, NB], DT.float32)
                nc.scalar.activation(lcol[:], colminT[:], AF.Ln)
                colexp = persist.tile([128, NB], DT.float32)
                nc.scalar.activation(
                    colexp[:], lcol[:], AF.Identity, scale=-1.0 / BETA, bias=refc[:]
                )
                colfin = persist.tile([128, NB], DT.float32)
                if ncopy:
                    nc.vector.tensor_tensor(
                        colfin[:], colexp[:], colminT2[:], ALU.min
                    )
                else:
                    nc.vector.tensor_copy(colfin[:], colexp[:])
            else:
                colfin = colminT

            rowr = persist.tile([128, NB], DT.float32)
            rowd = persist.tile([128, NB], DT.float32)
            nc.scalar.activation(rowr[:], rm[:], AF.Relu)
            nc.scalar.activation(rowd[:], rowr[:], AF.Sqrt)

            colr = persist.tile([128, NB], DT.float32)
            cold = persist.tile([128, NB], DT.float32)
            nc.scalar.activation(colr[:], colfin[:], AF.Relu)
            nc.scalar.activation(cold[:], colr[:], AF.Sqrt)

            junk1 = persist.tile([128, NB], DT.float32)
            outacc = persist.tile([128, 2], DT.float32)
            nc.vector.tensor_mul(junk1[:], rowd[:], w1t[:])
            nc.vector.tensor_reduce(
                outacc[:, 0:1], junk1[:], axis=mybir.AxisListType.X, op=ALU.add
            )
            junk2 = persist.tile([128, NB], DT.float32)
            nc.vector.tensor_mul(junk2[:], cold[:], w2t[:])
            nc.vector.tensor_reduce(
                outacc[:, 1:2], junk2[:], axis=mybir.AxisListType.X, op=ALU.add
            )
            nc.sync.dma_start(out_d[:], outacc[:])

    nc.compile()
    return nc


def _get_nc(repeat=1, parts=DEFAULT_PARTS):
    key = ("nc", repeat, parts)
    if key not in _CACHE:
        _CACHE[key] = _build_program(repeat, parts)
    return _CACHE[key]


def _make_in_maps(set1, set2, w1, w2):
    in_maps = []
    for b in range(N_CORES):
        in_maps.append(
            {
                "xt": np.ascontiguousarray(set1[b].T, dtype=np.float32),
                "yt": np.ascontiguousarray(set2[b].T, dtype=np.float32),
                "w1t": np.ascontiguousarray(
                    w1[b].reshape(NB, 128).T, dtype=np.float32
                ),
                "w2t": np.ascontiguousarray(
                    w2[b].reshape(NB, 128).T, dtype=np.float32
                ),
            }
        )
    return in_maps


def kernel(set1, set2, w1, w2):
    global LAST_RESULTS
    set1 = np.asarray(set1, dtype=np.float32)
    set2 = np.asarray(set2, dtype=np.float32)
    w1 = np.asarray(w1, dtype=np.float32)
    w2 = np.asarray(w2, dtype=np.float32)

    nc = _get_nc()
    in_maps = _make_in_maps(set1, set2, w1, w2)
    res = run_bass_kernel_spmd(nc, in_maps, core_ids=list(range(N_CORES)))
    LAST_RESULTS = res

    total = 0.0
    for core_out in res.results:
        total += float(core_out["out"].astype(np.float64).sum())
    return np.float32(total / 2.0)


# revision 15
# speedup vs baseline: 1.1348x; 1.0471x over previous
"""Weighted Chamfer loss on Trainium2 (8 NeuronCores, batch-parallel).

Problem (per batch element b of 8):
    dist[i, j] = || set1[b, i] - set2[b, j] ||_2            (4096 x 4096, C=128)
    total = (sum_i w1[b,i] * min_j dist + sum_j w2[b,j] * min_i dist) / 2

Sharding: one batch element per NeuronCore (pure data parallel, no
collectives); the 8 per-core partial sums are added on the host.

Per-core pipeline (units are [128 x 2048] fp32 PSUM tiles; 32 row blocks
x 2 column halves):
  PE  : ONE fp8 DoubleRow matmul per 512-col chunk computes
        m = x.y - x2/2 - y2/2 (= -d2/2) over a K=256 contraction:
        k-tile 0 = the 128 channels (e4m3-cast inputs), k-tile 1 = bake
        rows (3 fp8 residual rows per squared-norm + matching ones rows).
  exp mode (default): evacuation carries the row reduction.
    ACT : evacuates E-blocks as exp(-beta*(d2 - REF)) in bf16
          (Exp(scale=2*beta, bias=beta*REF) of the PSUM); its built-in
          sum accumulator emits per-unit softmin sums -> row-min comes
          out of the evac pass for free (rowmin = REF - ln(S)/beta).
    DVE : col pass only = tensor_tensor(max) of the (monotone) exps into
          colacc; for `dcopy` D-blocks DVE instead evacuates from PSUM
          via tensor_scalar(mult -2 -> fp16 d2) whose accum(min) is that
          half's row-min, and folds into a separate fp16 min-colacc
          (rebalances ACT vs DVE).
    Tail: PE transposes of both col accumulators + strided max/min
          reduces; Ln/Relu/Sqrt on ACT; masked merge of the two row/col
          conventions; weighted sums; host adds the 8 partials.
"""

import sys
from contextlib import ExitStack, nullcontext

import numpy as np

for _p in ("/opt/trn_rl_repo",):
    if _p not in sys.path:
        sys.path.insert(0, _p)

import concourse.bass as bass
import concourse.tile as tile
from concourse import bacc, masks, mybir
from concourse.bass_utils import run_bass_kernel_spmd

AF = mybir.ActivationFunctionType
ALU = mybir.AluOpType
DT = mybir.dt
PM = mybir.MatmulPerfMode

N_CORES = 8
N = 4096          # points per set per batch element
C = 128           # channels (= contraction dim = partition dim)
NB = N // 128     # 32 row blocks of x
UCOLS = 2048      # y columns per PSUM unit (half of PSUM)
NH = N // UCOLS   # 2 column halves
MMN = 512         # moving free dim per matmul (one fp32 PSUM bank)
NT = UCOLS // 128 # 16 transpose tiles per column half
NRES = 3          # fp8 residual rows per squared-norm bake

BETA = 0.75       # softmin sharpness (exp mode)
REF = 150.0       # softmin reference offset (exp mode)

_CACHE = {}
LAST_RESULTS = None  # BassKernelResults of the most recent run (for profiling)

DEFAULT_PARTS = "pe,act,dve,exp,dcopy8"


def _build_program(repeat=1, parts=DEFAULT_PARTS):
    en_act = "act" in parts
    en_dve = "dve" in parts
    en_col = "nocol" not in parts
    en_row = "norow" not in parts
    rowttr = "rowttr" in parts
    en_exp = "exp" in parts
    mmn = MMN
    ncopy = 0
    for p in parts.split(","):
        if p.startswith("dcopy"):
            ncopy = int(p[5:])
        if p.startswith("mmn"):
            mmn = int(p[3:])
    # dcopy blocks spread evenly through the 32 row blocks
    dset = set()
    if ncopy:
        step = NB // ncopy
        dset = {step // 2 + k * step for k in range(ncopy)}

    nc = bacc.Bacc(
        "TRN2", debug=False, target_bir_lowering=False, num_devices=N_CORES
    )
    xt_d = nc.dram_tensor("xt", [C, N], DT.float32, kind="ExternalInput").ap()
    yt_d = nc.dram_tensor("yt", [C, N], DT.float32, kind="ExternalInput").ap()
    w1t_d = nc.dram_tensor("w1t", [128, NB], DT.float32, kind="ExternalInput").ap()
    w2t_d = nc.dram_tensor("w2t", [128, NB], DT.float32, kind="ExternalInput").ap()
    out_d = nc.dram_tensor("out", [128, 2], DT.float32, kind="ExternalOutput").ap()

    with tile.TileContext(nc) as tc, ExitStack() as ctx:
        persist = ctx.enter_context(tc.tile_pool(name="persist", bufs=1))
        prep = ctx.enter_context(tc.tile_pool(name="prep", bufs=2))
        d2p = ctx.enter_context(tc.tile_pool(name="d2", bufs=6))
        d2fp = ctx.enter_context(tc.tile_pool(name="d2f", bufs=2))
        psum = ctx.enter_context(tc.tile_pool(name="psum", bufs=2, space="PSUM"))

        # ---------------- inputs ----------------
        xt = persist.tile([C, N], DT.float32)
        yt = persist.tile([C, N], DT.float32)
        nc.sync.dma_start(xt[:], xt_d[:])
        nc.sync.dma_start(yt[:], yt_d[:])
        w1t = persist.tile([128, NB], DT.float32)
        w2t = persist.tile([128, NB], DT.float32)
        nc.sync.dma_start(w1t[:], w1t_d[:])
        nc.sync.dma_start(w2t[:], w2t_d[:])

        # fused fp8 operands: k-tile 0 = channels, k-tile 1 = bake rows
        x8 = persist.tile([C, 2, N], DT.float8e4)
        y8 = persist.tile([C, 2, N], DT.float8e4)
        nc.vector.memset(x8[:], 0.0)
        nc.vector.memset(y8[:], 0.0)
        nc.vector.tensor_copy(x8[:, 0, :], xt[:])
        nc.vector.tensor_copy(y8[:, 0, :], yt[:])

        identity = persist.tile([128, 128], DT.float16)
        masks.make_identity(nc, identity[:])
        identb = persist.tile([128, 128], DT.bfloat16)
        nc.vector.tensor_copy(identb[:], identity[:])

        ones = persist.tile([C, 1], DT.float16)
        nc.gpsimd.memset(ones[:], 1.0)

        # exp-domain (bf16, max-acc) and d2-domain (fp16, min-acc) col accs
        colacc = persist.tile([128, N], DT.bfloat16 if en_exp else DT.float16)
        nc.gpsimd.memset(colacc[:], 0.0 if en_exp else 60000.0)
        colacc2 = None
        if en_exp and ncopy:
            colacc2 = persist.tile([128, N], DT.float16)
            nc.gpsimd.memset(colacc2[:], 60000.0)

        bref = persist.tile([128, 1], DT.float32)
        nc.gpsimd.memset(bref[:], BETA * REF)
        refc = persist.tile([128, 1], DT.float32)
        nc.gpsimd.memset(refc[:], REF)

        rm = persist.tile([128, NB], DT.float32)
        rmh0 = persist.tile([128, NB], DT.float32)
        rmh1 = persist.tile([128, NB], DT.float32)
        se0 = persist.tile([128, NB], DT.float32)
        se1 = persist.tile([128, NB], DT.float32)
        junk = None
        if not en_exp:
            junk = persist.tile([128, N], DT.float16)

        # D-column masks (exp mode): dm = 1 on dcopy columns, em = 1 - dm
        dm = persist.tile([128, NB], DT.float32)
        em = persist.tile([128, NB], DT.float32)
        nc.gpsimd.memset(dm[:], 0.0)
        nc.gpsimd.memset(em[:], 1.0)
        for b in sorted(dset):
            nc.gpsimd.memset(dm[:, b : b + 1], 1.0)
            nc.gpsimd.memset(em[:, b : b + 1], 0.0)

        # ---------------- squared norms -> fp8 residual bake rows ----------
        on8 = persist.tile([1, N], DT.float8e4)
        nc.vector.memset(on8[:], 1.0)
        for src, targ, pbase, oth in ((xt, x8, 0, y8), (yt, y8, NRES, x8)):
            sq = prep.tile([C, N], DT.float16, tag="sq", name=f"sq{pbase}")
            nc.scalar.activation(sq[:], src[:], AF.Square)
            v = prep.tile([1, N], DT.float32, tag="v", name=f"v{pbase}")
            for half in range(NH):
                ps = psum.tile(
                    [128, UCOLS], DT.float32, tag="unit", name=f"nps{pbase}{half}"
                )
                for k in range(UCOLS // MMN):
                    c0 = k * MMN
                    nc.tensor.matmul(
                        ps[0:1, c0 : c0 + MMN],
                        ones[:],
                        sq[:, half * UCOLS + c0 : half * UCOLS + c0 + MMN],
                        start=True,
                        stop=True,
                    )
                nc.scalar.activation(
                    v[0:1, half * UCOLS : (half + 1) * UCOLS],
                    ps[0:1, :],
                    AF.Identity,
                    scale=-0.5,
                )
            cur = v
            for k in range(NRES):
                r8 = prep.tile([1, N], DT.float8e4, tag="r8", name=f"r8{pbase}{k}")
                nc.vector.tensor_copy(r8[:], cur[:])
                nc.sync.dma_start(targ[pbase + k : pbase + k + 1, 1, :], r8[:])
                nc.sync.dma_start(oth[pbase + k : pbase + k + 1, 1, :], on8[:])
                if k < NRES - 1:
                    rb = prep.tile([1, N], DT.float32, tag="rb", name=f"rb{pbase}{k}")
                    nc.vector.tensor_copy(rb[:], r8[:])
                    nxt = prep.tile(
                        [1, N], DT.float32, tag="v", name=f"v{pbase}{k}"
                    )
                    nc.vector.tensor_tensor(nxt[:], cur[:], rb[:], ALU.subtract)
                    cur = nxt

        if not (en_dve and en_act and en_row):
            nc.gpsimd.memset(rm[:], 1.0)

        unroll = "unroll" in parts
        with (
            tc.For_i(0, repeat, 1)
            if (repeat > 1 and not unroll)
            else nullcontext()
        ):
          for _u in range(repeat if unroll else 1):
            # per-iteration neutral fill for partial-column accumulators
            if en_exp:
                nc.gpsimd.memset(se0[:], 1.0)
                nc.gpsimd.memset(se1[:], 1.0)
                if ncopy:
                    nc.gpsimd.memset(rmh0[:], 1.0)
                    nc.gpsimd.memset(rmh1[:], 1.0)
            # ---------------- main loop ----------------
            for b in range(NB):
                bcols = slice(b * 128, (b + 1) * 128)
                dvecopy = b in dset and en_dve
                if not en_exp:
                    if dvecopy:
                        d2 = d2fp.tile([128, N], DT.float16, tag="d2f", name="d2f")
                    else:
                        d2 = d2p.tile([128, N], DT.float16, tag="d2")
                for h in range(NH):
                    hcols = slice(h * UCOLS, (h + 1) * UCOLS)
                    if en_exp:
                        # per-unit tile: no intra-block false deps
                        if dvecopy:
                            d2u = d2fp.tile(
                                [128, UCOLS], DT.float16, tag="d2f", name="d2f"
                            )
                        else:
                            d2u = d2p.tile(
                                [128, UCOLS], DT.bfloat16, tag="d2"
                            )
                    ps = psum.tile([128, UCOLS], DT.float32, tag="unit")
                    for k in range(UCOLS // mmn):
                        c0 = k * mmn
                        nc.tensor.matmul(
                            ps[:, c0 : c0 + mmn],
                            x8[:, :, bcols],
                            y8[:, :, h * UCOLS + c0 : h * UCOLS + c0 + mmn],
                            start=True,
                            stop=True,
                            perf_mode=PM.DoubleRow,
                        )
                    rmh = rmh0 if h == 0 else rmh1
                    seh = se0 if h == 0 else se1
                    if dvecopy and en_exp:
                        nc.vector.tensor_scalar(
                            d2u[:],
                            ps[:],
                            -2.0,
                            None,
                            ALU.mult,
                            ALU.min,
                            accum_out=rmh[:, b : b + 1],
                        )
                        if en_col:
                            nc.vector.tensor_tensor(
                                colacc2[:, hcols],
                                d2u[:],
                                colacc2[:, hcols],
                                ALU.min,
                            )
                    elif dvecopy:
                        # DVE evac (PSUM fp32 -> SBUF fp16 d2, scale -2);
                        # accum is this half's row-min
                        nc.vector.tensor_scalar(
                            d2[:, hcols],
                            ps[:],
                            -2.0,
                            None,
                            ALU.mult,
                            ALU.min,
                            accum_out=rmh[:, b : b + 1],
                        )
                        if en_col:
                            nc.vector.tensor_tensor(
                                colacc2[:, hcols],
                                d2[:, hcols],
                                colacc2[:, hcols],
                                ALU.min,
                            )
                    elif en_act:
                        if en_exp:
                            # evac as exp(-beta*(d2-REF)); accum = softmin sum
                            nc.scalar.activation(
                                d2u[:],
                                ps[:],
                                AF.Exp,
                                bias=bref[:],
                                scale=2.0 * BETA,
                                accum_out=seh[:, b : b + 1] if en_row else None,
                            )
                            if en_dve and en_col:
                                nc.vector.tensor_tensor(
                                    colacc[:, hcols],
                                    d2u[:],
                                    colacc[:, hcols],
                                    ALU.max,
                                )
                        else:
                            nc.scalar.activation(
                                d2[:, hcols], ps[:], AF.Identity, scale=-2.0
                            )
                            if en_dve and en_col:
                                nc.vector.tensor_tensor(
                                    colacc[:, hcols],
                                    d2[:, hcols],
                                    colacc[:, hcols],
                                    ALU.min,
                                )
                if en_dve and en_act and en_row and not en_exp and not dvecopy:
                    if rowttr:
                        nc.vector.tensor_tensor(
                            junk[:, 0:UCOLS], d2[:, 0:UCOLS], d2[:, UCOLS:N], ALU.min
                        )
                        nc.vector.tensor_tensor(
                            junk[:, 0:1024], junk[:, 0:1024], junk[:, 1024:UCOLS], ALU.min
                        )
                        nc.vector.tensor_tensor(
                            junk[:, 0:512], junk[:, 0:512], junk[:, 512:1024], ALU.min
                        )
                        nc.vector.tensor_reduce(
                            rm[:, b : b + 1],
                            junk[:, 0:512],
                            axis=mybir.AxisListType.X,
                            op=ALU.min,
                        )
                    else:
                        nc.vector.tensor_scalar(
                            junk[:],
                            d2[:],
                            1.0,
                            None,
                            ALU.mult,
                            ALU.min,
                            accum_out=rm[:, b : b + 1],
                        )
                elif en_dve and en_act and en_row and not en_exp and dvecopy:
                    nc.vector.tensor_tensor(
                        rm[:, b : b + 1],
                        rmh0[:, b : b + 1],
                        rmh1[:, b : b + 1],
                        ALU.min,
                    )

            # ---------------- column-min tails ----------------
            # exp-domain: transpose colacc (bf16) + strided MAX reduce
            colminT = persist.tile([128, NB], DT.float32)
            for h in range(NH):
                pst = psum.tile(
                    [128, UCOLS],
                    DT.bfloat16 if en_exp else DT.float16,
                    tag="unit",
                    name="pst",
                )
                for t in range(NT):
                    nc.tensor.transpose(
                        pst[:, t * 128 : (t + 1) * 128],
                        colacc[:, h * UCOLS + t * 128 : h * UCOLS + (t + 1) * 128],
                        identb[:] if en_exp else identity[:],
                    )
                nc.vector.tensor_reduce(
                    colminT[:, h * NT : (h + 1) * NT],
                    pst[:].rearrange("p (t c) -> p t c", c=128),
                    axis=mybir.AxisListType.X,
                    op=ALU.max if en_exp else ALU.min,
                )
            colminT2 = None
            if en_exp and ncopy:
                colminT2 = persist.tile([128, NB], DT.float32)
                for h in range(NH):
                    pst2 = psum.tile(
                        [128, UCOLS], DT.float16, tag="unit", name="pst2"
                    )
                    for t in range(NT):
                        nc.tensor.transpose(
                            pst2[:, t * 128 : (t + 1) * 128],
                            colacc2[
                                :, h * UCOLS + t * 128 : h * UCOLS + (t + 1) * 128
                            ],
                            identity[:],
                        )
                    nc.vector.tensor_reduce(
                        colminT2[:, h * NT : (h + 1) * NT],
                        pst2[:].rearrange("p (t c) -> p t c", c=128),
                        axis=mybir.AxisListType.X,
                        op=ALU.min,
                    )

            # ---------------- tail ----------------
            if en_exp:
                # rows: REF - ln(se0+se1)/beta on E columns, min(rmh) on D
                sadd = persist.tile([128, NB], DT.float32)
                nc.vector.tensor_tensor(sadd[:], se0[:], se1[:], ALU.add)
                lrow = persist.tile([128, NB], DT.float32)
                nc.scalar.activation(lrow[:], sadd[:], AF.Ln)
                rmexp = persist.tile([128, NB], DT.float32)
                nc.scalar.activation(
                    rmexp[:], lrow[:], AF.Identity, scale=-1.0 / BETA, bias=refc[:]
                )
                if ncopy:
                    rmd = persist.tile([128, NB], DT.float32)
                    nc.vector.tensor_tensor(rmd[:], rmh0[:], rmh1[:], ALU.min)
                    t1 = persist.tile([128, NB], DT.float32)
                    t2 = persist.tile([128, NB], DT.float32)
                    nc.vector.tensor_mul(t1[:], rmexp[:], em[:])
                    nc.vector.tensor_mul(t2[:], rmd[:], dm[:])
                    nc.vector.tensor_tensor(rm[:], t1[:], t2[:], ALU.add)
                else:
                    nc.vector.tensor_copy(rm[:], rmexp[:])
                # cols: REF - ln(colmax)/beta, merged with fp16 min-colacc
                lcol = persist.tile([128, NB], DT.float32)
                nc.scalar.activation(lcol[:], colminT[:], AF.Ln)
                colexp = persist.tile([128, NB], DT.float32)
                nc.scalar.activation(
                    colexp[:], lcol[:], AF.Identity, scale=-1.0 / BETA, bias=refc[:]
                )
                colfin = persist.tile([128, NB], DT.float32)
                if ncopy:
                    nc.vector.tensor_tensor(
                        colfin[:], colexp[:], colminT2[:], ALU.min
                    )
                else:
                    nc.vector.tensor_copy(colfin[:], colexp[:])
            else:
                colfin = colminT

            rowr = persist.tile([128, NB], DT.float32)
            rowd = persist.tile([128, NB], DT.float32)
            nc.scalar.activation(rowr[:], rm[:], AF.Relu)
            nc.scalar.activation(rowd[:], rowr[:], AF.Sqrt)

            colr = persist.tile([128, NB], DT.float32)
            cold = persist.tile([128, NB], DT.float32)
            nc.scalar.activation(colr[:], colfin[:], AF.Relu)
            nc.scalar.activation(cold[:], colr[:], AF.Sqrt)

            junk1 = persist.tile([128, NB], DT.float32)
            outacc = persist.tile([128, 2], DT.float32)
            nc.vector.tensor_mul(junk1[:], rowd[:], w1t[:])
            nc.vector.tensor_reduce(
                outacc[:, 0:1], junk1[:], axis=mybir.AxisListType.X, op=ALU.add
            )
            junk2 = persist.tile([128, NB], DT.float32)
            nc.vector.tensor_mul(junk2[:], cold[:], w2t[:])
            nc.vector.tensor_reduce(
                outacc[:, 1:2], junk2[:], axis=mybir.AxisListType.X, op=ALU.add
            )
            nc.sync.dma_start(out_d[:], outacc[:])

    nc.compile()
    return nc


def _get_nc(repeat=1, parts=DEFAULT_PARTS):
    key = ("nc", repeat, parts)
    if key not in _CACHE:
        _CACHE[key] = _build_program(repeat, parts)
    return _CACHE[key]


def _make_in_maps(set1, set2, w1, w2):
    in_maps = []
    for b in range(N_CORES):
        in_maps.append(
            {
                "xt": np.ascontiguousarray(set1[b].T, dtype=np.float32),
                "yt": np.ascontiguousarray(set2[b].T, dtype=np.float32),
                "w1t": np.ascontiguousarray(
                    w1[b].reshape(NB, 128).T, dtype=np.float32
                ),
                "w2t": np.ascontiguousarray(
                    w2[b].reshape(NB, 128).T, dtype=np.float32
                ),
            }
        )
    return in_maps


def kernel(set1, set2, w1, w2):
    global LAST_RESULTS
    set1 = np.asarray(set1, dtype=np.float32)
    set2 = np.asarray(set2, dtype=np.float32)
    w1 = np.asarray(w1, dtype=np.float32)
    w2 = np.asarray(w2, dtype=np.float32)

    nc = _get_nc()
    in_maps = _make_in_maps(set1, set2, w1, w2)
    res = run_bass_kernel_spmd(nc, in_maps, core_ids=list(range(N_CORES)))
    LAST_RESULTS = res

    total = 0.0
    for core_out in res.results:
        total += float(core_out["out"].astype(np.float64).sum())
    return np.float32(total / 2.0)
